# revision 5
# baseline (speedup 1.0000x reference)
"""AtlasSpecializedLoss on 8 TRN2 NeuronCores — pure data parallel over B.
v2-verified fallback (HW 970us, rel err 8.1e-6)."""

import sys

for _p in ("/opt/trn_rl_repo", "/opt/pypackages"):
    if _p not in sys.path:
        sys.path.append(_p)

import numpy as np

import concourse.bass as bass
import concourse.bacc as bacc
from concourse import mybir
from concourse.tile import TileContext
from concourse.bass_utils import run_bass_kernel_spmd

F32 = mybir.dt.float32
AF = mybir.ActivationFunctionType
OP = mybir.AluOpType
AX = mybir.AxisListType

B, C, H, W = 4096, 10, 30, 30
PIX = H * W
NCORE = 8
BS = B // NCORE
P = 128
NT = BS // P
CH = 5

O_MPT, O_MCP, O_FOC, O_EDG, O_AFF, O_ROT, O_RFL = 0, 1, 2, 3, 4, 5, 6
O_PC = 10
O_RP = 20
O_SXP = 290
O_RT = 300
O_SXT = 600
OUTW = 640


def _bc(ap, pos, n):
    dims = list(ap.ap)
    dims.insert(pos + 1, [0, n])
    return bass.AP(tensor=ap.tensor, offset=ap.offset, ap=dims)


def build_graph() -> bass.Bass:
    nc = bacc.Bacc()
    pred = nc.declare_dram_parameter("pred", [BS, C * PIX], F32, isOutput=False)
    targ = nc.declare_dram_parameter("targ", [BS, C * PIX], F32, isOutput=False)
    ig = nc.declare_dram_parameter("ig", [BS, C * PIX], F32, isOutput=False)
    theta = nc.declare_dram_parameter("theta", [BS, 6], F32, isOutput=False)
    rot = nc.declare_dram_parameter("rot", [BS, 8], F32, isOutput=False)
    refl = nc.declare_dram_parameter("refl", [BS, 4], F32, isOutput=False)
    xmapd = nc.declare_dram_parameter("xmap", [1, PIX], F32, isOutput=False)
    kmapd = nc.declare_dram_parameter("kmap", [1, C], F32, isOutput=False)
    iotad = nc.declare_dram_parameter("iotac", [1, C], F32, isOutput=False)
    out = nc.declare_dram_parameter("out", [BS, OUTW], F32, isOutput=True)

    v = nc.vector
    a = nc.scalar
    g = nc.gpsimd

    with TileContext(nc) as tc:
        # Pin the combined ln+exp+copy+square activation table once; every
        # scalar op below stays inside it, so no per-switch 1283ns reloads.
        atl = mybir.InstLoadActFuncSet(
            name=nc.get_next_instruction_name(), ins=[], outs=[])
        atl.act_func_set_id = 6
        a_pin = nc.scalar
        a_pin.add_instruction(atl)
        with (
            tc.tile_pool(name="pr", bufs=1) as prp,
            tc.tile_pool(name="tg", bufs=2) as tgp,
            tc.tile_pool(name="igp", bufs=1) as igp,
            tc.tile_pool(name="big", bufs=1) as bigp,
            tc.tile_pool(name="per", bufs=1) as per,
            tc.tile_pool(name="scr", bufs=4) as scr,
            tc.tile_pool(name="outp", bufs=2) as outp,
            tc.tile_pool(name="tiny", bufs=8) as tiny,
            tc.tile_pool(name="cst", bufs=1) as cst,
        ):
            xmap = cst.tile([P, PIX], F32, tag="xmap")
            src = xmapd[0:1, :]
            nc.sync.dma_start(out=xmap, in_=bass.AP(
                tensor=src.tensor, offset=src.offset, ap=[[0, P], [1, PIX]]))
            kmap = cst.tile([P, C], F32, tag="kmap")
            src = kmapd[0:1, :]
            nc.sync.dma_start(out=kmap, in_=bass.AP(
                tensor=src.tensor, offset=src.offset, ap=[[0, P], [1, C]]))
            iotac = cst.tile([P, C], F32, tag="iotac")
            src = iotad[0:1, :]
            nc.sync.dma_start(out=iotac, in_=bass.AP(
                tensor=src.tensor, offset=src.offset, ap=[[0, P], [1, C]]))

            for t in range(NT):
                r0 = t * P

                pr = prp.tile([P, C, PIX], F32, tag="pr")
                nc.sync.dma_start(
                    out=pr[:, 0:CH, :],
                    in_=pred[r0:r0 + P, :CH * PIX].rearrange("p (c x) -> p c x", c=CH))
                nc.sync.dma_start(
                    out=pr[:, CH:C, :],
                    in_=pred[r0:r0 + P, CH * PIX:].rearrange("p (c x) -> p c x", c=CH))
                igt = igp.tile([P, C, PIX], F32, tag="igt")
                nc.sync.dma_start(
                    out=igt[:, 0:CH, :],
                    in_=ig[r0:r0 + P, :CH * PIX].rearrange("p (c x) -> p c x", c=CH))
                nc.sync.dma_start(
                    out=igt[:, CH:C, :],
                    in_=ig[r0:r0 + P, CH * PIX:].rearrange("p (c x) -> p c x", c=CH))

                ot = outp.tile([P, OUTW], F32, tag="ot")
                nc.gpsimd.memset(ot, 0.0)

                prf = pr.rearrange("p c x -> p (c x)")

                tidx = per.tile([P, PIX], F32, tag="tidx")
                ptacc = per.tile([P, PIX], F32, tag="ptacc")
                for cg in range(C // CH):
                    c0 = cg * CH
                    tg_t = tgp.tile([P, CH, PIX], F32, tag="tg")
                    nc.sync.dma_start(
                        out=tg_t,
                        in_=targ[r0:r0 + P, c0 * PIX:(c0 + CH) * PIX].rearrange(
                            "p (c x) -> p c x", c=CH))
                    tgf = tg_t.rearrange("p c x -> p (c x)")
                    v.tensor_reduce(
                        ot[:, O_RT + 30 * c0:O_RT + 30 * (c0 + CH)],
                        tg_t.rearrange("p c (h w) -> p c h w", w=W),
                        axis=AX.X, op=OP.add)
                    for cc in range(CH):
                        c = c0 + cc
                        if c == 1:
                            a.activation(tidx, tg_t[:, cc, :], AF.Copy,
                                         bias=0.0, scale=1.0)
                        elif c >= 2:
                            q = scr.tile([P, PIX], F32, tag="scr")
                            a.activation(q, tg_t[:, cc, :], AF.Copy,
                                         bias=0.0, scale=float(c))
                            g.tensor_tensor(tidx, tidx, q, OP.add)
                    bigt = bigp.tile([P, C * PIX], F32, tag="big")
                    v.tensor_mul(bigt[:, :CH * PIX], tgf,
                                 prf[:, c0 * PIX:(c0 + CH) * PIX])
                    v.tensor_add(bigt[:, 0:1800], bigt[:, 0:1800], bigt[:, 1800:3600])
                    v.tensor_add(bigt[:, 0:900], bigt[:, 0:900], bigt[:, 900:1800])
                    if cg == 0:
                        v.tensor_add(ptacc, bigt[:, 0:900], bigt[:, 3600:4500])
                    else:
                        q = scr.tile([P, PIX], F32, tag="scr")
                        v.tensor_add(q, bigt[:, 0:900], bigt[:, 3600:4500])
                        v.tensor_add(ptacc, ptacc, q)
                    g.tensor_tensor(tg_t, tg_t, _bc(xmap, 0, CH), OP.mult)
                    v.tensor_reduce(ot[:, O_SXT + c0:O_SXT + c0 + CH], tg_t,
                                    axis=AX.X, op=OP.add)

                mx = per.tile([P, PIX], F32, tag="mx")
                bigt = bigp.tile([P, C * PIX], F32, tag="big")
                v.tensor_max(bigt[:, 0:4500], prf[:, 0:4500], prf[:, 4500:9000])
                v.tensor_max(bigt[:, 0:1800], bigt[:, 0:1800], bigt[:, 1800:3600])
                v.tensor_max(mx, bigt[:, 0:900], bigt[:, 900:1800])
                v.tensor_max(mx, mx, bigt[:, 3600:4500])
                v.tensor_tensor(bigt.rearrange("p (c x) -> p c x", c=C), pr,
                                _bc(mx, 0, C), OP.is_equal)
                v.tensor_tensor(bigt.rearrange("p (c x) -> p c x", c=C),
                                bigt.rearrange("p (c x) -> p c x", c=C),
                                _bc(kmap, 1, PIX), OP.mult)
                v.tensor_max(bigt[:, 0:4500], bigt[:, 0:4500], bigt[:, 4500:9000])
                v.tensor_max(bigt[:, 0:1800], bigt[:, 0:1800], bigt[:, 1800:3600])
                v.tensor_max(bigt[:, 0:900], bigt[:, 0:900], bigt[:, 900:1800])
                v.tensor_max(bigt[:, 0:900], bigt[:, 0:900], bigt[:, 3600:4500])
                pidx = per.tile([P, PIX], F32, tag="pidx")
                a.activation(pidx, bigt[:, 0:900], AF.Copy, bias=1000.0, scale=-1.0)
                a.activation(pr, pr, AF.Exp)
                bigt = bigp.tile([P, C * PIX], F32, tag="big")
                ss = per.tile([P, PIX], F32, tag="ss")
                g.tensor_tensor(bigt[:, 0:4500], prf[:, 0:4500], prf[:, 4500:9000], OP.add)
                g.tensor_tensor(bigt[:, 0:1800], bigt[:, 0:1800], bigt[:, 1800:3600], OP.add)
                g.tensor_tensor(ss, bigt[:, 0:900], bigt[:, 900:1800], OP.add)
                g.tensor_tensor(ss, ss, bigt[:, 3600:4500], OP.add)
                lnss = per.tile([P, PIX], F32, tag="lnss")
                a.activation(lnss, ss, AF.Ln)
                rr = per.tile([P, PIX], F32, tag="rr")
                a.activation(rr, lnss, AF.Exp, bias=0.0, scale=-1.0)
                v.tensor_tensor(pr, pr, _bc(rr, 0, C), OP.mult)
                v.tensor_reduce(ot[:, O_PC:O_PC + C], pr, axis=AX.X, op=OP.add)

                eqv = scr.tile([P, PIX], F32, tag="scr")
                v.tensor_tensor(eqv, pidx, tidx, OP.is_equal)
                a.activation(eqv, eqv, AF.Copy, bias=0.0, scale=1.0,
                             accum_out=ot[:, O_MPT:O_MPT + 1])
                bigt = bigp.tile([P, C * PIX], F32, tag="big")
                mp = bigt.rearrange("p (c x) -> p c x", c=C)
                v.tensor_tensor(mp, _bc(pidx, 0, C), _bc(iotac, 1, PIX), OP.is_equal)
                v.tensor_reduce(ot[:, O_RP:O_RP + 270],
                                mp[:, 1:, :].rearrange("p c (h w) -> p c h w", w=W),
                                axis=AX.X, op=OP.add)
                g.tensor_tensor(igt, mp, igt, OP.mult)
                igtf = igt.rearrange("p c x -> p (c x)")
                a.activation(igtf, igtf, AF.Copy, bias=0.0, scale=1.0,
                             accum_out=ot[:, O_MCP:O_MCP + 1])
                g.tensor_tensor(mp[:, 1:, :], mp[:, 1:, :], _bc(xmap, 0, C - 1),
                                OP.mult)
                v.tensor_reduce(ot[:, O_SXP:O_SXP + C - 1], mp[:, 1:, :],
                                axis=AX.X, op=OP.add)

                thw = tidx.rearrange("p (h w) -> p h w", w=W)
                ee = scr.tile([P, PIX], F32, tag="scr")
                v.memset(ee, 0.0)
                dh = scr.tile([P, PIX], F32, tag="scr")
                v.tensor_tensor(dh[:, :870], tidx[:, 30:], tidx[:, :870], OP.not_equal)
                v.tensor_add(ee[:, 30:], ee[:, 30:], dh[:, :870])
                v.tensor_add(ee[:, :870], ee[:, :870], dh[:, :870])
                dw = scr.tile([P, PIX], F32, tag="scr")
                dwv = dw[:, :870].rearrange("p (h w) -> p h w", w=29)
                v.tensor_tensor(dwv, thw[:, :, 1:], thw[:, :, :29], OP.not_equal)
                eehw = ee.rearrange("p (h w) -> p h w", w=W)
                v.tensor_add(eehw[:, :, 1:], eehw[:, :, 1:], dwv)
                v.tensor_add(eehw[:, :, :29], eehw[:, :, :29], dwv)
                sw = per.tile([P, PIX], F32, tag="sw")
                v.tensor_scalar(sw, ee, 0.0, None, OP.is_gt)
                a.activation(sw, sw, AF.Copy, bias=1.0, scale=0.5)

                ce = scr.tile([P, PIX], F32, tag="scr")
                v.tensor_sub(ce, lnss, ptacc)
                pt = scr.tile([P, PIX], F32, tag="scr")
                a.activation(pt, ptacc, AF.Exp)
                v.tensor_mul(pt, pt, rr)
                a.activation(pt, pt, AF.Copy, bias=1.0, scale=-1.0)
                v.tensor_scalar_max(pt, pt, 1e-30)
                a.activation(pt, pt, AF.Ln)
                a.activation(pt, pt, AF.Exp, scale=1.4)
                v.tensor_mul(ce, ce, pt)
                v.tensor_mul(ce, ce, sw)
                a.activation(ce, ce, AF.Copy, bias=0.0, scale=1.0,
                             accum_out=ot[:, O_FOC:O_FOC + 1])

                def sobel(idxf, dst):
                    S = scr.tile([P, PIX], F32, tag="scr")
                    a.activation(S, idxf, AF.Copy, bias=0.0, scale=2.0)
                    g.tensor_tensor(S[:, 30:], S[:, 30:], idxf[:, :870], OP.add)
                    g.tensor_tensor(S[:, :870], S[:, :870], idxf[:, 30:], OP.add)
                    EX = scr.tile([P, PIX], F32, tag="scr")
                    vS = S.rearrange("p (h w) -> p h w", w=W)
                    vE = EX.rearrange("p (h w) -> p h w", w=W)
                    a.activation(vE[:, :, 0:1], vS[:, :, 1:2], AF.Copy, bias=0.0, scale=1.0)
                    a.activation(vE[:, :, 29:30], vS[:, :, 28:29], AF.Copy, bias=0.0, scale=-1.0)
                    v.tensor_sub(vE[:, :, 1:29], vS[:, :, 2:], vS[:, :, :28])
                    T = scr.tile([P, PIX], F32, tag="scr")
                    a.activation(T, idxf, AF.Copy, bias=0.0, scale=2.0)
                    vI = idxf.rearrange("p (h w) -> p h w", w=W)
                    vT = T.rearrange("p (h w) -> p h w", w=W)
                    g.tensor_tensor(vT[:, :, 1:], vT[:, :, 1:], vI[:, :, :29], OP.add)
                    g.tensor_tensor(vT[:, :, :29], vT[:, :, :29], vI[:, :, 1:], OP.add)
                    EY = scr.tile([P, PIX], F32, tag="scr")
                    a.activation(EY[:, :30], T[:, 30:60], AF.Copy, bias=0.0, scale=1.0)
                    a.activation(EY[:, 870:], T[:, 840:870], AF.Copy, bias=0.0, scale=-1.0)
                    v.tensor_sub(EY[:, 30:870], T[:, 60:], T[:, :840])
                    a.square(EX, EX)
                    a.square(EY, EY)
                    v.tensor_add(EX, EX, EY)
                    v.tensor_scalar_max(EX, EX, 1e-30)
                    a.activation(EX, EX, AF.Ln)
                    a.activation(dst, EX, AF.Exp, scale=0.5)

                pe = per.tile([P, PIX], F32, tag="pe")
                te = scr.tile([P, PIX], F32, tag="scr")
                sobel(pidx, pe)
                sobel(tidx, te)
                v.tensor_sub(pe, pe, te)
                a.activation(pe, pe, AF.Square, accum_out=ot[:, O_EDG:O_EDG + 1])

                th = tiny.tile([P, 6], F32, tag="th")
                nc.sync.dma_start(out=th, in_=theta[r0:r0 + P, :])
                a.square(th, th)
                ssum = tiny.tile([P, 2], F32, tag="ssum")
                v.tensor_reduce(ssum[:, 0:1],
                                th.rearrange("p (r k) -> p r k", k=3)[:, :, 0:2],
                                axis=AX.XY, op=OP.add)
                v.tensor_reduce(ssum[:, 1:2],
                                th.rearrange("p (r k) -> p r k", k=3)[:, :, 2:3],
                                axis=AX.XY, op=OP.add)
                v.tensor_scalar_max(ssum, ssum, 1e-30)
                a.activation(ssum, ssum, AF.Ln)
                a.activation(ssum, ssum, AF.Exp, scale=0.5)
                q = tiny.tile([P, 1], F32, tag="q1")
                a.activation(q, ssum[:, 1:2], AF.Copy, bias=0.0, scale=0.1)
                v.tensor_add(ot[:, O_AFF:O_AFF + 1], ssum[:, 0:1], q)

                def entropy(src2, n, dst, tagp):
                    lgt = tiny.tile([P, n], F32, tag=tagp)
                    nc.sync.dma_start(out=lgt, in_=src2[r0:r0 + P, :])
                    m8 = tiny.tile([P, 1], F32, tag=tagp + "m")
                    v.tensor_reduce(m8, lgt, axis=AX.X, op=OP.max)
                    nm = tiny.tile([P, 1], F32, tag=tagp + "n")
                    a.activation(nm, m8, AF.Copy, bias=0.0, scale=-1.0)
                    z8 = tiny.tile([P, n], F32, tag=tagp + "z")
                    v.tensor_scalar(z8, lgt, nm, None, OP.add)
                    e8 = tiny.tile([P, n], F32, tag=tagp + "e")
                    a.activation(e8, lgt, AF.Exp, bias=nm)
                    s8 = tiny.tile([P, 1], F32, tag=tagp + "s")
                    v.tensor_reduce(s8, e8, axis=AX.X, op=OP.add)
                    dot = tiny.tile([P, 1], F32, tag=tagp + "d")
                    dsk = tiny.tile([P, n], F32, tag=tagp + "k")
                    v.tensor_mul(dsk, e8, z8)
                    v.tensor_reduce(dot, dsk, axis=AX.X, op=OP.add)
                    r8 = tiny.tile([P, 1], F32, tag=tagp + "r")
                    v.reciprocal(r8, s8)
                    v.tensor_mul(dot, dot, r8)
                    a.activation(s8, s8, AF.Ln)
                    v.tensor_sub(dst, s8, dot)

                entropy(rot, 8, ot[:, O_ROT:O_ROT + 1], "ro")
                entropy(refl, 4, ot[:, O_RFL:O_RFL + 1], "rf")

                nc.sync.dma_start(out=out[r0:r0 + P, :], in_=ot)
    nc.finalize()
    return nc


_GRAPH = None


def _get_graph():
    global _GRAPH
    if _GRAPH is None:
        _GRAPH = build_graph()
    return _GRAPH


def run_device(inputs: dict, trace: bool = False):
    pred = np.asarray(inputs["pred_output"], np.float32).reshape(B, C * PIX)
    targ = np.asarray(inputs["target_output"], np.float32).reshape(B, C * PIX)
    igrid = np.asarray(inputs["input_grid"], np.float32).reshape(B, C * PIX)
    theta = np.asarray(inputs["theta"], np.float32).reshape(B, 6)
    rot = np.asarray(inputs["rotation_logits"], np.float32).reshape(B, 8)
    refl = np.asarray(inputs["reflection_logits"], np.float32).reshape(B, 4)

    xmap = np.tile(np.arange(W, dtype=np.float32), H).reshape(1, PIX)
    kmap = (1000.0 - np.arange(C, dtype=np.float32)).reshape(1, C)
    iotac = np.arange(C, dtype=np.float32).reshape(1, C)

    in_maps = []
    for i in range(NCORE):
        s = slice(i * BS, (i + 1) * BS)
        in_maps.append({
            "pred": np.ascontiguousarray(pred[s]),
            "targ": np.ascontiguousarray(targ[s]),
            "ig": np.ascontiguousarray(igrid[s]),
            "theta": np.ascontiguousarray(theta[s]),
            "rot": np.ascontiguousarray(rot[s]),
            "refl": np.ascontiguousarray(refl[s]),
            "xmap": xmap,
            "kmap": kmap,
            "iotac": iotac,
        })
    res = run_bass_kernel_spmd(_get_graph(), in_maps, core_ids=list(range(NCORE)),
                               trace=trace)
    outs = np.concatenate([r["out"] for r in res.results], axis=0)
    return outs, res


def assemble(outs: np.ndarray) -> np.ndarray:
    o = outs.astype(np.float64)
    npix = float(B * PIX)
    match_pt, match_cp = o[:, O_MPT], o[:, O_MCP]
    spatial_focal = o[:, O_FOC].sum() / npix
    exact = match_pt == PIX
    exact_count = exact.sum()
    exact_bonus = -exact.mean() * 7.0
    transform = (match_cp == PIX).mean() * 0.2
    affine = o[:, O_AFF].mean() * 0.4
    rotation = o[:, O_ROT].mean() * 0.3
    reflection = o[:, O_RFL].mean() * 0.3
    edge = o[:, O_EDG].sum() / npix * 0.3

    pc = o[:, O_PC:O_PC + 10]
    rows_p = o[:, O_RP:O_RP + 270].reshape(B, 9, 30)
    sx_p = o[:, O_SXP:O_SXP + 9]
    rows_t = o[:, O_RT:O_RT + 300].reshape(B, 10, 30)
    sx_t = o[:, O_SXT + 1:O_SXT + 10]

    tc_full = rows_t.sum(2)
    pcn = pc / (pc.sum(1, keepdims=True) + 1e-8)
    tcn = tc_full / (tc_full.sum(1, keepdims=True) + 1e-8)
    cbal = ((pcn - tcn) ** 2).mean() * 0.2

    hh = np.arange(H, dtype=np.float64)

    def centers(rows, sx):
        cnt = rows.sum(2)
        cy = (rows * hh).sum(2) / np.maximum(cnt, 1.0)
        cx = sx / np.maximum(cnt, 1.0)
        return cy, cx, cnt > 0

    cyp, cxp, prp = centers(rows_p, sx_p)
    cyt, cxt, prt = centers(rows_t[:, 1:, :], sx_t)
    PI, PJ = np.triu_indices(9, 1)
    NP = PI.shape[0]

    def compact(cy, cx, pres):
        d = np.sqrt((cy[:, PI] - cy[:, PJ]) ** 2 + (cx[:, PI] - cx[:, PJ]) ** 2)
        vv = pres[:, PI] & pres[:, PJ]
        rank = np.cumsum(vv, axis=1) - 1
        slot = np.where(vv, rank, NP)
        comp = np.zeros((B, NP + 1))
        np.put_along_axis(comp, slot, d, axis=1)
        return comp[:, :NP], vv.sum(1)

    dpc, n_p = compact(cyp, cxp, prp)
    dtc, n_t = compact(cyt, cxt, prt)
    m = np.minimum(n_p, n_t)
    use = np.arange(NP)[None, :] < m[:, None]
    sq = (((dpc - dtc) ** 2) * use).sum(1)
    geo_b = np.where(m > 0, sq / np.maximum(m, 1), 0.0)
    geo = geo_b.sum() / B * 0.5

    total = (spatial_focal + transform + affine + rotation + reflection
             + geo + edge + cbal + exact_bonus)
    return np.array([total, spatial_focal, transform, exact_bonus, exact_count,
                     affine, rotation, reflection, geo, edge, cbal], np.float32)


def kernel(**inputs) -> np.ndarray:
    outs, _ = run_device(inputs, trace=False)
    return assemble(outs)

